# revision 8
# baseline (speedup 1.0000x reference)
"""Trainium2 Bass kernel for nn_Cross_Attention (gnn message passing).

Self-contained: accepts FULL inputs, shards data-parallel over the M query
points across 8 NeuronCores, runs a Bass/Tile kernel per core, gathers the
full [M, C] output.

Reference math:
    qp = (q+q_pos)@Wqk + bqk ; kp = (k+k_pos)@Wqk + bqk
    v  = value@Wv + bv
    e  = relu((qp[:,None,:] - kp[idx])@Wg1 + bg1)@Wg2 + bg2
    e  = where(mask, -1e12, e); attn = softmax(e, axis=1)
    out = einsum('mkc,mkc->mc', attn, v) @ Wt + bt

Kernel algebra / layout (v4 — host-expanded streaming, software-pipelined):
  * bqk cancels in qp - kp[idx]; W1 = Wqk@Wg1 composed on host, so layer 1 is
    (sq - sk[idx])@W1 with sq = q+q_pos, sk = k+k_pos.
  * The host expands the k-NN gather into a channel-major bf16 edge stream
    kgT[128, EH]: partitions 0-63 carry sk[idx]^T for query half A, 64-127
    for half B ("dup" layout).  The device just streams it: no dma_gather,
    no XBAR transposes, no idx upload.  v/value and sq use the same layout.
  * Mask via count-correction instead of a -1e12 matmul pass: the host sets
    each masked edge's kg column := its query's sq column (so the L1 PSUM
    cancels exactly and p_masked = exp(relu(bg1)@Wg2 + bg2) =: expc, a
    per-channel constant), zeroes the masked v columns (numerator), and
    ships per-query masked counts; the device subtracts cnt x expc from Z
    with one rank-2 matmul per chunk.  This removes the per-edge mask matmul.
  * L1 is blockdiag(-W1)@kg + blockdiag(W1)@sq_rep accumulated in PSUM.
  * Engine assignment: PE all matmuls; ACT relu+exp; DVE p*(v@Wv), the
    grouped-16 reduces (bf16 accumulators), and the normalize tail.  The
    issue order is software-pipelined one sub-tile deep so the PE never
    waits on ACT (keeps the tensor engine at its max p-state).
  * normalize after aggregation: num = sum_k P*(v@Wv), Z = sum_k P - cnt*expc,
    res = num/Z; out = res@Wt + (bv@Wt + bt), channel-major, host untransposes.
"""
import sys

sys.path.insert(0, "/opt/trn_rl_repo")
if "/root/.axon_site" not in sys.path:
    sys.path.insert(0, "/root/.axon_site")

import numpy as np
import ml_dtypes

import concourse.bass as bass
import concourse.tile as tile
from concourse import bacc, mybir
from concourse.bass_utils import run_bass_kernel_spmd

BF16 = mybir.dt.bfloat16
F32 = mybir.dt.float32
AF = mybir.ActivationFunctionType
ALU = mybir.AluOpType

N_CORES = 8


class Cfg:
    def __init__(self, M=65536, N=65536, K=16, C=64, chunk_cols=2048, sub=512):
        self.M, self.N, self.K, self.C = M, N, K, C
        self.MC = M // N_CORES          # queries per core
        self.MH = self.MC // 2          # queries per half
        self.EH = self.MH * K           # edge columns per half
        self.CHUNK = chunk_cols         # edge columns per chunk
        self.NCHUNK = self.EH // self.CHUNK
        self.SUB = sub
        self.NSUB = self.CHUNK // sub
        assert self.EH % self.CHUNK == 0 and self.CHUNK % sub == 0
        assert sub % K == 0 and (self.CHUNK // K) % 128 == 0


def build_nc(cfg: Cfg):
    c = cfg
    nc = bacc.Bacc(None)
    dp = nc.declare_dram_parameter

    kg_ext = dp("kgT", [128, c.EH], BF16, isOutput=False)
    v_ext = dp("vT", [128, c.EH], BF16, isOutput=False)
    sq_ext = dp("sqT", [128, c.MH], BF16, isOutput=False)
    cnt_ext = dp("cntr", [2, c.MH], BF16, isOutput=False)
    wn_ext = dp("Wn", [128, 128], BF16, isOutput=False)
    w1q_ext = dp("W1bd", [128, 128], BF16, isOutput=False)
    wg2_ext = dp("Wg2bd", [128, 128], BF16, isOutput=False)
    wv_ext = dp("Wvbd", [128, 128], BF16, isOutput=False)
    wt_ext = dp("Wtbd", [128, 128], BF16, isOutput=False)
    nec_ext = dp("negexpc2", [2, 128], BF16, isOutput=False)
    bg1_ext = dp("bg1d", [128, 1], F32, isOutput=False)
    bg2_ext = dp("bg2d", [128, 1], F32, isOutput=False)
    bto_ext = dp("btod", [128, 1], F32, isOutput=False)
    out_ext = dp("out_cm", [128, c.MH], F32, isOutput=True)

    mq = c.CHUNK // c.K                 # queries completed per chunk
    nq = c.SUB // c.K                   # queries completed per sub

    with tile.TileContext(nc) as tc:
        with tc.tile_pool(name="const", bufs=1) as constp, \
             tc.tile_pool(name="chunk", bufs=3) as chp, \
             tc.tile_pool(name="subt", bufs=4) as subp, \
             tc.tile_pool(name="hps", bufs=2, space="PSUM") as hps, \
             tc.tile_pool(name="eps", bufs=2, space="PSUM") as eps, \
             tc.tile_pool(name="vps", bufs=3, space="PSUM") as vps, \
             tc.tile_pool(name="ops", bufs=1, space="PSUM") as ops:

            # ---- constants ----
            wn = constp.tile([128, 128], BF16)
            w1q = constp.tile([128, 128], BF16)
            wg2 = constp.tile([128, 128], BF16)
            wv = constp.tile([128, 128], BF16)
            wt = constp.tile([128, 128], BF16)
            nec = constp.tile([2, 128], BF16)
            bg1 = constp.tile([128, 1], F32)
            bg2 = constp.tile([128, 1], F32)
            bto = constp.tile([128, 1], F32)
            sq = constp.tile([128, c.MH], BF16)
            cnt = constp.tile([2, c.MH], BF16)
            for t, e in ((wn, wn_ext), (w1q, w1q_ext), (wg2, wg2_ext),
                         (wv, wv_ext), (wt, wt_ext), (nec, nec_ext),
                         (bg1, bg1_ext), (bg2, bg2_ext), (bto, bto_ext),
                         (sq, sq_ext), (cnt, cnt_ext)):
                nc.sync.dma_start(out=t[:], in_=e[:])

            T = c.NCHUNK * c.NSUB
            st = {}          # per-sub tiles in flight
            ch = {}          # per-chunk tiles in flight

            def sq_rep_ap(t):
                m0 = t * nq
                sqs = sq[:, m0:m0 + nq]
                return bass.AP(tensor=sqs.tensor, offset=sqs.offset,
                               ap=[sqs.ap[0], sqs.ap[1], [0, c.K]])

            for t in range(T + 3):
                ci, si = divmod(t, c.NSUB)

                # ---- chunk-entry: stream loads + accumulators ----
                if t < T and si == 0:
                    cl = slice(ci * c.CHUNK, (ci + 1) * c.CHUNK)
                    kg = chp.tile([128, c.CHUNK], BF16, tag="kg")
                    nc.sync.dma_start(out=kg[:], in_=kg_ext[:, cl])
                    vt = chp.tile([128, c.CHUNK], BF16, tag="vt")
                    nc.sync.dma_start(out=vt[:], in_=v_ext[:, cl])
                    z_t = chp.tile([128, mq], BF16, tag="zt")
                    n_t = chp.tile([128, mq], BF16, tag="nt")
                    ch[ci] = {"kg": kg, "vt": vt, "z": z_t, "n": n_t}

                # ---- stage 1 (sub t): L1 matmuls + v-projection + relu ----
                if t < T:
                    cc = ch[ci]
                    cs = slice(si * c.SUB, (si + 1) * c.SUB)
                    h_ps = hps.tile([128, c.SUB], F32)
                    nc.tensor.matmul(out=h_ps[:], lhsT=wn[:],
                                     rhs=cc["kg"][:, cs], start=True, stop=False)
                    nc.tensor.matmul(out=h_ps[:], lhsT=w1q[:],
                                     rhs=sq_rep_ap(t), start=False, stop=True)
                    vp_ps = vps.tile([128, c.SUB], F32)
                    nc.tensor.matmul(out=vp_ps[:], lhsT=wv[:],
                                     rhs=cc["vt"][:, cs], start=True, stop=True)
                    h_t = subp.tile([128, c.SUB], BF16, tag="h")
                    nc.scalar.activation(out=h_t[:], in_=h_ps[:], func=AF.Relu,
                                         bias=bg1[:, 0:1])
                    st[t] = (h_t, vp_ps, cc, si)

                # ---- stage 2 (sub t-1): L2 matmul, exp, p*vp, reduces ----
                u = t - 1
                if 0 <= u < T:
                    h_t, vp_ps, cc, si_u = st[u]
                    e_ps = eps.tile([128, c.SUB], F32)
                    nc.tensor.matmul(out=e_ps[:], lhsT=wg2[:], rhs=h_t[:],
                                     start=True, stop=True)
                    p_t = subp.tile([128, c.SUB], BF16, tag="p")
                    nc.scalar.activation(out=p_t[:], in_=e_ps[:], func=AF.Exp,
                                         bias=bg2[:, 0:1])
                    pv_t = subp.tile([128, c.SUB], BF16, tag="pv")
                    nc.vector.tensor_tensor(out=pv_t[:], in0=p_t[:],
                                            in1=vp_ps[:], op=ALU.mult)
                    zc = slice(si_u * nq, (si_u + 1) * nq)
                    with nc.allow_low_precision(reason="16-group z/n sums"):
                        nc.vector.tensor_reduce(
                            out=cc["z"][:, zc],
                            in_=p_t[:].rearrange("p (m k) -> p m k", k=c.K),
                            axis=mybir.AxisListType.X, op=ALU.add)
                        nc.vector.tensor_reduce(
                            out=cc["n"][:, zc],
                            in_=pv_t[:].rearrange("p (m k) -> p m k", k=c.K),
                            axis=mybir.AxisListType.X, op=ALU.add)
                    del st[u]

                # ---- chunk tails, skewed behind the main pipeline ----
                if t >= c.NSUB + 1 and si == 1:
                    tci = ci - 1
                    cc = ch[tci]
                    corr_ps = ops.tile([128, mq], F32, tag="o")
                    nc.tensor.matmul(out=corr_ps[:], lhsT=nec[:],
                                     rhs=cnt[:, tci * mq:(tci + 1) * mq],
                                     start=True, stop=True)
                    z32 = subp.tile([128, mq], F32, tag="z32")
                    nc.vector.tensor_tensor(out=z32[:], in0=cc["z"][:],
                                            in1=corr_ps[:], op=ALU.add)
                    nc.vector.reciprocal_approx_fast(out=z32[:], in_=z32[:])
                    res_t = subp.tile([128, mq], BF16, tag="res")
                    nc.vector.tensor_tensor(out=res_t[:], in0=cc["n"][:],
                                            in1=z32[:], op=ALU.mult)
                    cc["res"] = res_t
                if t >= c.NSUB + 2 and si == 2:
                    tci = ci - 1
                    cc = ch[tci]
                    o_ps = ops.tile([128, mq], F32, tag="o")
                    nc.tensor.matmul(out=o_ps[:], lhsT=wt[:], rhs=cc["res"][:],
                                     start=True, stop=True)
                    outc = subp.tile([128, mq], F32, tag="outc")
                    nc.scalar.activation(out=outc[:], in_=o_ps[:],
                                         func=AF.Identity, bias=bto[:, 0:1])
                    nc.sync.dma_start(
                        out=out_ext[:, tci * mq:(tci + 1) * mq],
                        in_=outc[:])
                    del ch[tci]
    nc.finalize()
    return nc


def blockdiag(w):
    bd = np.zeros((128, 128), np.float32)
    bd[:64, :64] = w
    bd[64:, 64:] = w
    return bd.astype(ml_dtypes.bfloat16)


def prep_weights(Wqk, Wv, Wg1, Wg2, Wt, bg1, bg2, bto):
    bf = ml_dtypes.bfloat16
    W1 = (Wqk @ Wg1).astype(np.float32)
    # expc replicates the device arithmetic for a masked edge (kg == sq):
    # h = relu(bg1) stored bf16 by ACT, e = h@Wg2(bf16) + bg2 (f32 accum),
    # p = exp(e) stored bf16.
    h_m = np.maximum(bg1, 0.0).astype(bf).astype(np.float32)
    e_m = h_m @ Wg2.astype(bf).astype(np.float32) + bg2
    expc = np.exp(e_m).astype(bf).astype(np.float32)      # [C]
    nec = np.zeros((2, 128), np.float32)
    nec[0, :64] = -expc
    nec[1, 64:] = -expc
    return {
        "Wn": blockdiag(-W1), "W1bd": blockdiag(W1),
        "Wg2bd": blockdiag(Wg2), "Wvbd": blockdiag(Wv), "Wtbd": blockdiag(Wt),
        "negexpc2": nec.astype(bf),
        "bg1d": np.tile(bg1.astype(np.float32), 2).reshape(128, 1),
        "bg2d": np.tile(bg2.astype(np.float32), 2).reshape(128, 1),
        "btod": np.tile(bto.astype(np.float32), 2).reshape(128, 1),
    }


def prep_core_inputs(cfg: Cfg, core, skT, sqT_all, vT_all, mask, idx, wdict):
    """Per-core input dict.  skT is the [C, N] bf16 transposed key table."""
    c = cfg
    s = core * c.MC

    ic = idx[s:s + c.MC].reshape(c.MC * c.K)
    kgT = np.empty((128, c.EH), ml_dtypes.bfloat16)
    kgT[0:64] = skT[:, ic[:c.EH]]
    kgT[64:128] = skT[:, ic[c.EH:]]

    sqT = np.concatenate(
        [sqT_all[:, s:s + c.MH], sqT_all[:, s + c.MH:s + c.MC]], axis=0)
    vT = np.concatenate(
        [vT_all[:, s * c.K:s * c.K + c.EH],
         vT_all[:, s * c.K + c.EH:s * c.K + 2 * c.EH]], axis=0)

    # mask handling: poison kg (=> exact L1 cancellation), zero v, count rows
    mc = np.asarray(mask[s:s + c.MC]).reshape(c.MC, c.K)
    cntq = mc.sum(axis=1).astype(np.float32)
    cnt2 = np.stack([cntq[:c.MH], cntq[c.MH:]], axis=0)\
        .astype(ml_dtypes.bfloat16)
    mflat = mc.reshape(c.MC * c.K)
    colsA = np.nonzero(mflat[:c.EH])[0]
    colsB = np.nonzero(mflat[c.EH:])[0]
    kgT[0:64, colsA] = sqT[0:64, colsA // c.K]
    kgT[64:128, colsB] = sqT[64:128, colsB // c.K]
    vT[0:64, colsA] = 0
    vT[64:128, colsB] = 0

    m = dict(wdict)
    m.update({"kgT": kgT, "vT": vT, "sqT": sqT, "cntr": cnt2})
    return m


_NC_CACHE = {}


def run(cfg: Cfg, inputs, trace=False):
    q = np.asarray(inputs["q"], np.float32)
    k = np.asarray(inputs["k"], np.float32)
    value = np.asarray(inputs["value"], np.float32)
    q_pos = np.asarray(inputs["q_pos"], np.float32)
    k_pos = np.asarray(inputs["k_pos"], np.float32)
    mask = np.asarray(inputs["mask"])
    kni = np.asarray(inputs["knearest_idx"])
    idx = kni.reshape(kni.shape[0], -1, cfg.K)[1]
    Wqk = np.asarray(inputs["Wqk"], np.float32)
    Wv = np.asarray(inputs["Wv"], np.float32)
    Wg1 = np.asarray(inputs["Wg1"], np.float32)
    Wg2 = np.asarray(inputs["Wg2"], np.float32)
    Wt = np.asarray(inputs["Wt"], np.float32)
    bg1 = np.asarray(inputs["bg1"], np.float32)
    bg2 = np.asarray(inputs["bg2"], np.float32)
    bv = np.asarray(inputs["bv"], np.float32)
    bt = np.asarray(inputs["bt"], np.float32)
    bto = bv @ Wt + bt

    key = (cfg.M, cfg.N, cfg.CHUNK, cfg.SUB)
    if key not in _NC_CACHE:
        _NC_CACHE[key] = build_nc(cfg)
    nc = _NC_CACHE[key]

    bf = ml_dtypes.bfloat16
    skT = np.ascontiguousarray((k + k_pos).astype(bf).T)      # [64, N]
    sqT_all = np.ascontiguousarray((q + q_pos).astype(bf).T)  # [64, M]
    vT_all = np.ascontiguousarray(
        value.reshape(cfg.M * cfg.K, cfg.C).astype(bf).T)     # [64, M*K]

    wdict = prep_weights(Wqk, Wv, Wg1, Wg2, Wt, bg1, bg2, bto)
    in_maps = [prep_core_inputs(cfg, core, skT, sqT_all, vT_all, mask, idx,
                                wdict) for core in range(N_CORES)]

    res = run_bass_kernel_spmd(nc, in_maps, core_ids=list(range(N_CORES)),
                               trace=trace)
    out = np.empty((cfg.M, cfg.C), np.float32)
    for core in range(N_CORES):
        s = core * cfg.MC
        oc = res.results[core]["out_cm"]
        out[s:s + cfg.MH] = oc[0:64].T
        out[s + cfg.MH:s + cfg.MC] = oc[64:128].T
    return out, res


def kernel(**inputs) -> np.ndarray:
    cfg = Cfg()
    out, _ = run(cfg, inputs)
    return out.astype(np.float32)
